# revision 13
# baseline (speedup 1.0000x reference)
"""Trainium2 Bass kernel for the quantized ResNet bottleneck block.

Data-parallel over batch: 64 images -> 8 cores x 8 images.

Precision strategy: the final bfp quant rounds to delta = 2^(e-6) per
(pixel, 32-channel block); any pre-quant perturbation below delta/2 moves an
output by at most ONE quant step (= the error the exact kernel already shows
from fp32-order effects), so bf16-level approximations (2^-9) are free in
max-rel-error terms.  Verified empirically on CPU: conv1 2-term / conv2
hi-only / conv3 hi-only / bf16 residual all land at rel = one step (0.0094).

Per-core pipeline (channel-major [C_part, pix] for conv1/conv2, pixel-major
for layer 3):
  conv1: 2 bf16 matmuls per 128-k-tile (w1hi*xh + w1lo*xh), xh = bf16(x+b3).
  bn1+relu: ScalarE activation (per-partition scale/bias).
  bfp quant: vector transpose-reduce -> per-(pixel,block) max; ONE int32 AND
         masks it to 2^e (the 2^-6 delta scaling is folded into the DVE
         constants s0/s1; relu'd inputs make the 1e-24 clamp unnecessary:
         a zero block max yields in1=0 and the op then outputs exactly 0);
         broadcast-read v.transpose replicates 2^e across the 32 channels;
         one fused DVE op does round/clip/rescale via the magic-constant
         trick (fp32-internal datapath).
  conv2: 3x3 via 9 shifted-window matmuls on a zero-padded buffer, single
         bf16 weights (activations are exact in bf16).
  conv3: pixel-major (lhsT = a2), single bf16 weight w3hi; residual added via
         one identity matmul against host-transposed bf16 xTh; ScalarE copies
         PSUM->SBUF bf16 with fused ReLU; block max via two bf16 2x-mode
         tensor_max levels + a short reduce; one DVE quant per 2-tile emit.
  output: bf16 (quantized values are exact in bf16), transposed on the host.

bn3 trick: the kernel consumes xh ~= x + bn3_beta instead of x, so the
residual add needs no extra bias op; conv1's bias is corrected by
-inv1 * (w1q @ bn3_beta) on the host.

The last group's quant2 is split into 1024+544-pixel chunks so the layer-3
tail can start earlier.
"""
import numpy as np
import ml_dtypes
from contextlib import ExitStack

import concourse.bass as bass
import concourse.bacc as bacc
import concourse.tile as tile
from concourse import mybir
from concourse.bass_utils import run_bass_kernel_spmd

F32 = mybir.dt.float32
BF16 = mybir.dt.bfloat16
I32 = mybir.dt.int32
AL = mybir.AluOpType
AFT = mybir.ActivationFunctionType

# ---------------- custom DVE op: fused bfp round/clip/rescale ---------------
# out = min(max(in0 + in1*s0, in1*s0), in1*s1) - in1*s0
# in1 = 2^e (the exponent-masked block max); s0 = 1.5*2^17 so in1*s0 is the
# magic constant 1.5*2^(17+e) whose ulp is delta = 2^(e-6): the fp32 add
# rounds in0 to the delta grid (round-half-even); the clips implement relu
# and the 127 cap (s1 = s0 + 127/64, exact); the subtract is exact
# (Sterbenz).  The DVE computes in fp32 internally regardless of dtypes.
import concourse.dve_ops as dve_ops
from concourse.dve_spec import Spec, Src0, Src1, C0, C1, minn, maxx

EXP_S0 = 196608.0          # 1.5 * 2^17
EXP_S1 = 196609.984375     # s0 + 127/64, exactly representable

def _bfp_ref(in0, in1, s0, s1, imm2):
    lo = in1 * s0
    return (np.minimum(np.maximum(in0 + lo, lo), in1 * s1) - lo).astype(np.float32)

BFP_QUANT_ANT = dve_ops.DveOp(
    "BFP_QUANT_ANT",
    Spec(
        body=minn(maxx(Src0 + Src1 * C0, Src1 * C0), Src1 * C1) - Src1 * C0,
        reference=_bfp_ref,
    ),
    subdim=False,
    uops_sha={"v3": "09229989be91bde3", "v4": "701a1ee7014b78c5"},
)

def _register_bfp_op():
    if "BFP_QUANT_ANT" not in dve_ops._SUB_OPCODE_FOR_NAME:
        dve_ops.OPS.append(BFP_QUANT_ANT)
        dve_ops.CUSTOM_DVE_SPECS["BFP_QUANT_ANT"] = BFP_QUANT_ANT.spec
        dve_ops._SUB_OPCODE_FOR_NAME["BFP_QUANT_ANT"] = (
            dve_ops._CUSTOM_DVE_ROW_BASE + len(dve_ops.OPS) - 1)

_register_bfp_op()

# ---------------- geometry (hardcoded for this problem) ---------------------
N_IMG = 8          # images per core
CIN = 512
WID = 128
H = W = 28
HW = H * W         # 784
PIX = N_IMG * HW   # 6272
PADH = PADW = 30
NT392 = 392        # conv N-tile (14 rows)
GRP = 1568         # quant group = 2 images


def build_nc():
    nc = bacc.Bacc()

    xh = nc.declare_dram_parameter("xh", [N_IMG, CIN, HW], BF16, False)
    xTh = nc.declare_dram_parameter("xTh", [PIX, CIN], BF16, False)
    ident = nc.declare_dram_parameter("ident", [128, 128], BF16, False)
    w1h = nc.declare_dram_parameter("w1h", [CIN, WID], BF16, False)
    w1l = nc.declare_dram_parameter("w1l", [CIN, WID], BF16, False)
    w2h = nc.declare_dram_parameter("w2h", [9, WID, WID], BF16, False)
    w3h = nc.declare_dram_parameter("w3h", [WID, CIN], BF16, False)
    inv1 = nc.declare_dram_parameter("inv1", [WID, 1], F32, False)
    bet1 = nc.declare_dram_parameter("bet1", [WID, 1], F32, False)
    inv2 = nc.declare_dram_parameter("inv2", [WID, 1], F32, False)
    bet2 = nc.declare_dram_parameter("bet2", [WID, 1], F32, False)
    outT = nc.declare_dram_parameter("outT", [PIX, CIN], BF16, True)

    with tile.TileContext(nc) as tc, ExitStack() as ctx:
        wp = ctx.enter_context(tc.tile_pool(name="wp", bufs=1))
        big = ctx.enter_context(tc.tile_pool(name="big", bufs=1))
        xsl = ctx.enter_context(tc.tile_pool(name="xsl", bufs=8))
        ygp = ctx.enter_context(tc.tile_pool(name="ygp", bufs=3))
        dsm = ctx.enter_context(tc.tile_pool(name="dsm", bufs=4))
        y3p = ctx.enter_context(tc.tile_pool(name="y3p", bufs=3))
        tp = ctx.enter_context(tc.tile_pool(name="tp", bufs=3))
        xt3 = ctx.enter_context(tc.tile_pool(name="xt3", bufs=4))
        stage = ctx.enter_context(tc.tile_pool(name="stage", bufs=3))
        pp = ctx.enter_context(tc.tile_pool(name="pp", bufs=2, space="PSUM"))
        p3p = ctx.enter_context(tc.tile_pool(name="p3p", bufs=6, space="PSUM"))

        # ---- params in (critical-path order) ----
        w1hsb = wp.tile([128, 4, WID], BF16)
        nc.scalar.dma_start(w1hsb[:], w1h[:].rearrange("(k c) o -> c k o", c=128))
        w1lsb = wp.tile([128, 4, WID], BF16)
        nc.scalar.dma_start(w1lsb[:], w1l[:].rearrange("(k c) o -> c k o", c=128))
        bn1s = wp.tile([128, 1], F32); nc.scalar.dma_start(bn1s[:], inv1[:])
        bn1b = wp.tile([128, 1], F32); nc.scalar.dma_start(bn1b[:], bet1[:])
        # ---- activations / residual state ----
        a1pad = big.tile([128, N_IMG, PADH, PADW], BF16)
        nc.gpsimd.memset(a1pad[:].rearrange("p n h w -> p (n h w)").bitcast(I32), 0)
        a2 = big.tile([128, PIX], BF16)

        # per-image input slabs [c, k, q] bf16 (hi only)
        xslabs = {}

        def emit_xload(n, split=False):
            th = xsl.tile([128, 4, HW], BF16, tag="xh")
            src_ = xh[n].rearrange("(k c) q -> c k q", c=128)
            if split:
                for q0, q1 in ((0, NT392), (NT392, HW)):
                    for k0 in (0, 2):
                        nc.sync.dma_start(th[:, k0:k0+2, q0:q1],
                                          src_[:, k0:k0+2, q0:q1])
            else:
                nc.sync.dma_start(th[:], src_)
            xslabs[n] = th

        def emit_expmask(rm):
            # rm (f32 AP, in place): rm = 2^floor(log2(rm)); rm=+0 stays +0.
            nc.vector.tensor_scalar(rm.bitcast(I32), rm.bitcast(I32),
                                    0x7F800000, None, op0=AL.bitwise_and)

        # ================= emit functions =================
        taps = [(dy, dx) for dy in range(3) for dx in range(3)]

        def emit_l1(g, chunks=((0, GRP),)):
            ygrp = ygp.tile([128, GRP], F32, tag="ygrp")
            for si in range(4):
                n = 2 * g + si // 2
                q0 = NT392 * (si % 2)
                th = xslabs[n]
                pst = pp.tile([128, CIN], F32, tag="cp")
                ps = pst[:, :NT392]
                i = 0
                for k in range(4):
                    for lhsT in (w1hsb, w1lsb):
                        nc.tensor.matmul(ps[:], lhsT[:, k, :], th[:, k, q0:q0+NT392],
                                         start=(i == 0), stop=(i == 7))
                        i += 1
                nc.scalar.activation(ygrp[:, si*NT392:(si+1)*NT392], ps[:], AFT.Relu,
                                     bias=bn1b[:], scale=bn1s[:])
            for s0, npx in chunks:
                nb = npx // 32
                rmax = dsm.tile([128, 49], F32, tag="rmax")
                nc.vector.tensor_reduce(
                    rmax[:, :nb],
                    ygrp[:, s0:s0+npx].rearrange("p (b j) -> p b j", b=nb, j=32),
                    axis=mybir.AxisListType.X, op=AL.max, apply_transpose=True)
                emit_expmask(rmax[:, :nb])
                dcm = dsm.tile([128, GRP], F32, tag="dcm")
                nc.vector.transpose(dcm[:, :npx],
                                    rmax[:, :nb].unsqueeze(2).broadcast_to([128, nb, 32]))
                # per-image, row-aligned pieces (chunk bounds are multiples of 224)
                a, end = s0, s0 + npx
                while a < end:
                    im = a // HW
                    b = min(end, (im + 1) * HW)
                    r0, r1 = (a - im * HW) // 28, (b - im * HW) // 28
                    nc.vector._custom_dve(
                        BFP_QUANT_ANT,
                        out=a1pad[:, 2*g+im, 1+r0:1+r1, 1:29],
                        in0=ygrp[:, a:b],
                        in1=dcm[:, a-s0:b-s0],
                        s0=EXP_S0, s1=EXP_S1,
                    )
                    a = b

        def emit_l2(g, chunks=((0, GRP),)):
            ygrp = ygp.tile([128, GRP], F32, tag="y2grp")
            for si in range(4):
                n = 2 * g + si // 2
                h0 = 14 * (si % 2)
                pst = pp.tile([128, CIN], F32, tag="cp")
                ps = pst[:, :NT392]
                for t, (dy, dx) in enumerate(taps):
                    rhs = a1pad[:, n, h0+dy:h0+dy+14, dx:dx+28]
                    nc.tensor.matmul(ps[:], w2hsb[:, t, :], rhs,
                                     start=(t == 0), stop=(t == 8))
                nc.scalar.activation(ygrp[:, si*NT392:(si+1)*NT392], ps[:], AFT.Relu,
                                     bias=bn2b[:], scale=bn2s[:])
            for s0, npx in chunks:
                nb = npx // 32
                rmax = dsm.tile([128, 49], F32, tag="rmax")
                nc.vector.tensor_reduce(
                    rmax[:, :nb],
                    ygrp[:, s0:s0+npx].rearrange("p (b j) -> p b j", b=nb, j=32),
                    axis=mybir.AxisListType.X, op=AL.max, apply_transpose=True)
                emit_expmask(rmax[:, :nb])
                dcm = dsm.tile([128, GRP], F32, tag="dcm")
                nc.vector.transpose(dcm[:, :npx],
                                    rmax[:, :nb].unsqueeze(2).broadcast_to([128, nb, 32]))
                nc.vector._custom_dve(
                    BFP_QUANT_ANT,
                    out=a2[:, 2*g*HW + s0:2*g*HW + s0 + npx],
                    in0=ygrp[:, s0:s0+npx],
                    in1=dcm[:, :npx],
                    s0=EXP_S0, s1=EXP_S1,
                )

        def emit_l3(t0, gn):
            nf = gn * CIN
            na = nf // 32          # 32-channel blocks in this emit
            xh3 = xt3.tile([128, 5 * CIN], BF16, tag="xh3")
            nc.sync.dma_start(xh3[:, :nf].rearrange("p (j c) -> p j c", j=gn, c=CIN),
                              xTh[128*t0:128*t0 + 128*gn, :].rearrange("(j p) c -> p j c", p=128))
            y3b = y3p.tile([128, 5 * CIN], BF16, tag="y3b")
            for j in range(gn):
                ps3 = p3p.tile([128, CIN], F32, tag="c3g")
                a2t = a2[:, 128*(t0+j):128*(t0+j+1)]
                nc.tensor.matmul(ps3[:], a2t, w3hsb[:], start=True, stop=False)
                nc.tensor.matmul(ps3[:], identsb[:], xh3[:, j*CIN:(j+1)*CIN],
                                 start=False, stop=True)
                nc.scalar.activation(y3b[:, j*CIN:(j+1)*CIN], ps3[:], AFT.Relu)
            # block max: two bf16 2x-mode tensor_max levels, then a short reduce
            yv = y3b[:, :nf].rearrange("p (a k) -> p a k", k=32)
            t1 = tp.tile([128, 1280], BF16, tag="t1")
            t1v = t1[:].rearrange("p (a k) -> p a k", k=16)[:, :na, :]
            nc.vector.tensor_max(t1v, yv[:, :, 0:16], yv[:, :, 16:32])
            t2 = tp.tile([128, 640], BF16, tag="t2")
            t2v = t2[:].rearrange("p (a k) -> p a k", k=8)[:, :na, :]
            nc.vector.tensor_max(t2v, t1v[:, :, 0:8], t1v[:, :, 8:16])
            rm3 = dsm.tile([128, 80], F32, tag="rm3")
            nc.vector.tensor_reduce(rm3[:, :na], t2v,
                                    axis=mybir.AxisListType.X, op=AL.max)
            emit_expmask(rm3[:, :na])
            o3 = stage.tile([128, 5 * CIN], BF16, tag="o3")
            nc.vector._custom_dve(
                BFP_QUANT_ANT,
                out=o3[:, :nf].rearrange("p (a k) -> p a k", k=32),
                in0=yv,
                in1=rm3[:, :na].unsqueeze(2).broadcast_to([128, na, 32]),
                s0=EXP_S0, s1=EXP_S1,
            )
            nc.sync.dma_start(outT[128*t0:128*t0 + 128*gn, :].rearrange("(j p) c -> p j c", p=128),
                              o3[:, :nf].rearrange("p (j c) -> p j c", j=gn, c=CIN))

        # ================= interleaved schedule =================
        emit_xload(0, split=True)
        for n in range(1, 4):
            emit_xload(n)
        # non-critical params go behind the first input slabs in the sync queue
        w2hsb = wp.tile([128, 9, WID], BF16)
        nc.sync.dma_start(w2hsb[:], w2h[:].rearrange("t c o -> c t o"))
        bn2s = wp.tile([128, 1], F32); nc.sync.dma_start(bn2s[:], inv2[:])
        bn2b = wp.tile([128, 1], F32); nc.sync.dma_start(bn2b[:], bet2[:])
        w3hsb = wp.tile([128, CIN], BF16)
        nc.sync.dma_start(w3hsb[:], w3h[:])
        identsb = wp.tile([128, 128], BF16)
        nc.sync.dma_start(identsb[:], ident[:])
        emit_l1(0, chunks=((0, 1120), (1120, 448)))
        emit_xload(4); emit_xload(5)
        emit_l1(1)
        emit_l2(0)
        emit_xload(6); emit_xload(7)
        emit_l1(2)
        for t0 in (0, 4, 8):        # tiles 0-11: needs quant2(0) only
            emit_l3(t0, 4)
        emit_l2(1)
        emit_l1(3)
        for t0 in (12, 16, 20):     # tiles 12-23: needs quant2(1)
            emit_l3(t0, 4)
        emit_l2(2)
        for t0 in (24, 28, 32):     # tiles 24-35: needs quant2(2)
            emit_l3(t0, 4)
        emit_l2(3, chunks=((0, 1024), (1024, 544)))
        for t0 in (36, 40):         # tiles 36-43: needs quant2(3) chunk A
            emit_l3(t0, 4)
        emit_l3(44, 5)              # tiles 44-48: needs quant2(3) chunk B

    nc.finalize()
    return nc


# ---------------- host-side parameter prep ---------------------------------
def _w_quant_np(w, blk=32):
    O, I, kh, kw = w.shape
    wb = w.reshape(O, I // blk, blk, kh, kw)
    alpha = np.maximum(np.abs(wb).max(axis=2, keepdims=True) / np.float32(127.0),
                       np.float32(1e-24)).astype(np.float32)
    q = (np.round(wb / alpha) * alpha).astype(np.float32)
    return q.reshape(O, I, kh, kw)


def _bn_fold(g, b, m, v):
    inv = (g / np.sqrt(v + np.float32(1e-5))).astype(np.float32)
    beta = (b - m * inv).astype(np.float32)
    return inv, beta


def _split_bf16(a):
    hi = a.astype(ml_dtypes.bfloat16)
    lo = (a - hi.astype(np.float32)).astype(ml_dtypes.bfloat16)
    return hi, lo


_NC_CACHE = {}

def kernel(x, w1, w2, w3,
           bn1_g, bn1_b, bn1_m, bn1_v,
           bn2_g, bn2_b, bn2_m, bn2_v,
           bn3_g, bn3_b, bn3_m, bn3_v,
           _want_trace=False):
    x = np.asarray(x, np.float32)
    w1q = _w_quant_np(np.asarray(w1, np.float32))
    w2q = _w_quant_np(np.asarray(w2, np.float32))
    w3q = _w_quant_np(np.asarray(w3, np.float32))
    inv1, bet1 = _bn_fold(*[np.asarray(a, np.float32) for a in (bn1_g, bn1_b, bn1_m, bn1_v)])
    inv2, bet2 = _bn_fold(*[np.asarray(a, np.float32) for a in (bn2_g, bn2_b, bn2_m, bn2_v)])
    inv3, bet3 = _bn_fold(*[np.asarray(a, np.float32) for a in (bn3_g, bn3_b, bn3_m, bn3_v)])

    # bn3 beta folded into the residual input; conv1 bias corrected for it
    xb3 = (x + bet3[None, :, None, None]).astype(np.float32)
    K = (w1q[:, :, 0, 0].astype(np.float64) @ bet3.astype(np.float64))
    bet1c = (bet1.astype(np.float64) - inv1.astype(np.float64) * K).astype(np.float32)

    w1T = np.ascontiguousarray(w1q[:, :, 0, 0].T)                     # [512, 128]
    w2T = np.ascontiguousarray(w2q.transpose(2, 3, 1, 0).reshape(9, WID, WID))
    w3f = (w3q[:, :, 0, 0] * inv3[:, None]).astype(np.float32)
    w3T = np.ascontiguousarray(w3f.T)                                 # [128, 512]

    w1hT, w1lT = _split_bf16(w1T)
    w2hT = w2T.astype(ml_dtypes.bfloat16)
    w3hT = w3T.astype(ml_dtypes.bfloat16)

    xb3v = xb3.reshape(64, CIN, HW)
    xh_ = xb3v.astype(ml_dtypes.bfloat16)                             # [64, 512, 784]
    # residual uses the same hi rounding, transposed to pixel-major
    xTh_ = np.ascontiguousarray(xh_.transpose(0, 2, 1))               # [64, 784, 512]

    if "nc" not in _NC_CACHE:
        _NC_CACHE["nc"] = build_nc()
    nc = _NC_CACHE["nc"]

    shared = dict(
        w1h=w1hT, w1l=w1lT, w2h=w2hT, w3h=w3hT,
        ident=np.eye(128, dtype=ml_dtypes.bfloat16),
        inv1=inv1.reshape(WID, 1), bet1=bet1c.reshape(WID, 1),
        inv2=inv2.reshape(WID, 1), bet2=bet2.reshape(WID, 1),
    )
    in_maps = []
    for c in range(8):
        m = dict(shared)
        m["xh"] = np.ascontiguousarray(xh_[8*c:8*(c+1)])
        m["xTh"] = np.ascontiguousarray(xTh_[8*c:8*(c+1)].reshape(PIX, CIN))
        in_maps.append(m)

    res = run_bass_kernel_spmd(nc, in_maps, list(range(8)), trace=_want_trace)
    out = np.empty((64, CIN, H, W), np.float32)
    for c in range(8):
        oT = res.results[c]["outT"].astype(np.float32).reshape(N_IMG, HW, CIN)
        out[8*c:8*(c+1)] = oT.transpose(0, 2, 1).reshape(N_IMG, CIN, H, W)
    if _want_trace:
        return out, res
    return out


# revision 14
# speedup vs baseline: 1.0076x; 1.0076x over previous
"""Trainium2 Bass kernel for the quantized ResNet bottleneck block.

Data-parallel over batch: 64 images -> 8 cores x 8 images.

Precision strategy: the final bfp quant rounds to delta = 2^(e-6) per
(pixel, 32-channel block); any pre-quant perturbation below delta/2 moves an
output by at most ONE quant step (= the error the exact kernel already shows
from fp32-order effects), so bf16-level approximations (2^-9) are free in
max-rel-error terms.  Verified empirically on CPU: conv1 2-term / conv2
hi-only / conv3 hi-only / bf16 residual all land at rel = one step (0.0094).

Per-core pipeline (channel-major [C_part, pix] for conv1/conv2, pixel-major
for layer 3):
  conv1: 2 bf16 matmuls per 128-k-tile (w1hi*xh + w1lo*xh), xh = bf16(x+b3).
  bn1+relu: ScalarE activation (per-partition scale/bias).
  bfp quant: vector transpose-reduce -> per-(pixel,block) max; ONE int32 AND
         masks it to 2^e (the 2^-6 delta scaling is folded into the DVE
         constants s0/s1; relu'd inputs make the 1e-24 clamp unnecessary:
         a zero block max yields in1=0 and the op then outputs exactly 0);
         broadcast-read v.transpose replicates 2^e across the 32 channels;
         one fused DVE op does round/clip/rescale via the magic-constant
         trick (fp32-internal datapath).
  conv2: 3x3 via 9 shifted-window matmuls on a zero-padded buffer, single
         bf16 weights (activations are exact in bf16).
  conv3: pixel-major (lhsT = a2), single bf16 weight w3hi; residual added via
         one identity matmul against host-transposed bf16 xTh; ScalarE copies
         PSUM->SBUF bf16 with fused ReLU; block max via two bf16 2x-mode
         tensor_max levels + a short reduce; one DVE quant per 2-tile emit.
  output: bf16 (quantized values are exact in bf16), transposed on the host.

bn3 trick: the kernel consumes xh ~= x + bn3_beta instead of x, so the
residual add needs no extra bias op; conv1's bias is corrected by
-inv1 * (w1q @ bn3_beta) on the host.

The last group's quant2 is split into 1024+544-pixel chunks so the layer-3
tail can start earlier.
"""
import numpy as np
import ml_dtypes
from contextlib import ExitStack

import concourse.bass as bass
import concourse.bacc as bacc
import concourse.tile as tile
from concourse import mybir
from concourse.bass_utils import run_bass_kernel_spmd

F32 = mybir.dt.float32
BF16 = mybir.dt.bfloat16
I32 = mybir.dt.int32
AL = mybir.AluOpType
AFT = mybir.ActivationFunctionType

# ---------------- custom DVE op: fused bfp round/clip/rescale ---------------
# out = min(max(in0 + in1*s0, in1*s0), in1*s1) - in1*s0
# in1 = 2^e (the exponent-masked block max); s0 = 1.5*2^17 so in1*s0 is the
# magic constant 1.5*2^(17+e) whose ulp is delta = 2^(e-6): the fp32 add
# rounds in0 to the delta grid (round-half-even); the clips implement relu
# and the 127 cap (s1 = s0 + 127/64, exact); the subtract is exact
# (Sterbenz).  The DVE computes in fp32 internally regardless of dtypes.
import concourse.dve_ops as dve_ops
from concourse.dve_spec import Spec, Src0, Src1, C0, C1, minn, maxx

EXP_S0 = 196608.0          # 1.5 * 2^17
EXP_S1 = 196609.984375     # s0 + 127/64, exactly representable

def _bfp_ref(in0, in1, s0, s1, imm2):
    lo = in1 * s0
    return (np.minimum(np.maximum(in0 + lo, lo), in1 * s1) - lo).astype(np.float32)

BFP_QUANT_ANT = dve_ops.DveOp(
    "BFP_QUANT_ANT",
    Spec(
        body=minn(maxx(Src0 + Src1 * C0, Src1 * C0), Src1 * C1) - Src1 * C0,
        reference=_bfp_ref,
    ),
    subdim=False,
    uops_sha={"v3": "09229989be91bde3", "v4": "701a1ee7014b78c5"},
)

def _register_bfp_op():
    if "BFP_QUANT_ANT" not in dve_ops._SUB_OPCODE_FOR_NAME:
        dve_ops.OPS.append(BFP_QUANT_ANT)
        dve_ops.CUSTOM_DVE_SPECS["BFP_QUANT_ANT"] = BFP_QUANT_ANT.spec
        dve_ops._SUB_OPCODE_FOR_NAME["BFP_QUANT_ANT"] = (
            dve_ops._CUSTOM_DVE_ROW_BASE + len(dve_ops.OPS) - 1)

_register_bfp_op()

# ---------------- geometry (hardcoded for this problem) ---------------------
N_IMG = 8          # images per core
CIN = 512
WID = 128
H = W = 28
HW = H * W         # 784
PIX = N_IMG * HW   # 6272
PADH = PADW = 30
NT392 = 392        # conv N-tile (14 rows)
GRP = 1568         # quant group = 2 images


def build_nc():
    nc = bacc.Bacc()

    xh = nc.declare_dram_parameter("xh", [N_IMG, CIN, HW], BF16, False)
    xTh = nc.declare_dram_parameter("xTh", [PIX, CIN], BF16, False)
    ident = nc.declare_dram_parameter("ident", [128, 128], BF16, False)
    w1h = nc.declare_dram_parameter("w1h", [CIN, WID], BF16, False)
    w1l = nc.declare_dram_parameter("w1l", [CIN, WID], BF16, False)
    w2h = nc.declare_dram_parameter("w2h", [9, WID, WID], BF16, False)
    w3h = nc.declare_dram_parameter("w3h", [WID, CIN], BF16, False)
    inv1 = nc.declare_dram_parameter("inv1", [WID, 1], F32, False)
    bet1 = nc.declare_dram_parameter("bet1", [WID, 1], F32, False)
    inv2 = nc.declare_dram_parameter("inv2", [WID, 1], F32, False)
    bet2 = nc.declare_dram_parameter("bet2", [WID, 1], F32, False)
    outT = nc.declare_dram_parameter("outT", [PIX, CIN], BF16, True)

    with tile.TileContext(nc) as tc, ExitStack() as ctx:
        wp = ctx.enter_context(tc.tile_pool(name="wp", bufs=1))
        big = ctx.enter_context(tc.tile_pool(name="big", bufs=1))
        xsl = ctx.enter_context(tc.tile_pool(name="xsl", bufs=8))
        ygp = ctx.enter_context(tc.tile_pool(name="ygp", bufs=3))
        dsm = ctx.enter_context(tc.tile_pool(name="dsm", bufs=4))
        y3p = ctx.enter_context(tc.tile_pool(name="y3p", bufs=3))
        tp = ctx.enter_context(tc.tile_pool(name="tp", bufs=3))
        xt3 = ctx.enter_context(tc.tile_pool(name="xt3", bufs=4))
        stage = ctx.enter_context(tc.tile_pool(name="stage", bufs=3))
        pp = ctx.enter_context(tc.tile_pool(name="pp", bufs=2, space="PSUM"))
        p3p = ctx.enter_context(tc.tile_pool(name="p3p", bufs=6, space="PSUM"))

        # ---- params in (critical-path order) ----
        w1hsb = wp.tile([128, 4, WID], BF16)
        nc.scalar.dma_start(w1hsb[:], w1h[:].rearrange("(k c) o -> c k o", c=128))
        w1lsb = wp.tile([128, 4, WID], BF16)
        nc.scalar.dma_start(w1lsb[:], w1l[:].rearrange("(k c) o -> c k o", c=128))
        bn1s = wp.tile([128, 1], F32); nc.scalar.dma_start(bn1s[:], inv1[:])
        bn1b = wp.tile([128, 1], F32); nc.scalar.dma_start(bn1b[:], bet1[:])
        # ---- activations / residual state ----
        a1pad = big.tile([128, N_IMG, PADH, PADW], BF16)
        nc.gpsimd.memset(a1pad[:].rearrange("p n h w -> p (n h w)").bitcast(I32), 0)
        a2 = big.tile([128, PIX], BF16)

        # per-image input slabs [c, k, q] bf16 (hi only)
        xslabs = {}

        def emit_xload(n, split=False):
            th = xsl.tile([128, 4, HW], BF16, tag="xh")
            src_ = xh[n].rearrange("(k c) q -> c k q", c=128)
            if split:
                for q0, q1 in ((0, NT392), (NT392, HW)):
                    for k0 in (0, 2):
                        nc.sync.dma_start(th[:, k0:k0+2, q0:q1],
                                          src_[:, k0:k0+2, q0:q1])
            else:
                nc.sync.dma_start(th[:], src_)
            xslabs[n] = th

        def emit_expmask(rm):
            # rm (f32 AP, in place): rm = 2^floor(log2(rm)); rm=+0 stays +0.
            nc.vector.tensor_scalar(rm.bitcast(I32), rm.bitcast(I32),
                                    0x7F800000, None, op0=AL.bitwise_and)

        # ================= emit functions =================
        taps = [(dy, dx) for dy in range(3) for dx in range(3)]

        def emit_l1(g, chunks=((0, GRP),)):
            ygrp = ygp.tile([128, GRP], F32, tag="ygrp")
            for si in range(4):
                n = 2 * g + si // 2
                q0 = NT392 * (si % 2)
                th = xslabs[n]
                pst = pp.tile([128, CIN], F32, tag="cp")
                ps = pst[:, :NT392]
                i = 0
                for k in range(4):
                    for lhsT in (w1hsb, w1lsb):
                        nc.tensor.matmul(ps[:], lhsT[:, k, :], th[:, k, q0:q0+NT392],
                                         start=(i == 0), stop=(i == 7))
                        i += 1
                nc.scalar.activation(ygrp[:, si*NT392:(si+1)*NT392], ps[:], AFT.Relu,
                                     bias=bn1b[:], scale=bn1s[:])
            for s0, npx in chunks:
                nb = npx // 32
                rmax = dsm.tile([128, 49], F32, tag="rmax")
                nc.vector.tensor_reduce(
                    rmax[:, :nb],
                    ygrp[:, s0:s0+npx].rearrange("p (b j) -> p b j", b=nb, j=32),
                    axis=mybir.AxisListType.X, op=AL.max, apply_transpose=True)
                emit_expmask(rmax[:, :nb])
                dcm = dsm.tile([128, GRP], F32, tag="dcm")
                nc.vector.transpose(dcm[:, :npx],
                                    rmax[:, :nb].unsqueeze(2).broadcast_to([128, nb, 32]))
                # per-image, row-aligned pieces (chunk bounds are multiples of 224)
                a, end = s0, s0 + npx
                while a < end:
                    im = a // HW
                    b = min(end, (im + 1) * HW)
                    r0, r1 = (a - im * HW) // 28, (b - im * HW) // 28
                    nc.vector._custom_dve(
                        BFP_QUANT_ANT,
                        out=a1pad[:, 2*g+im, 1+r0:1+r1, 1:29],
                        in0=ygrp[:, a:b],
                        in1=dcm[:, a-s0:b-s0],
                        s0=EXP_S0, s1=EXP_S1,
                    )
                    a = b

        def emit_l2(g, chunks=((0, GRP),)):
            ygrp = ygp.tile([128, GRP], F32, tag="y2grp")
            for si in range(4):
                n = 2 * g + si // 2
                h0 = 14 * (si % 2)
                pst = pp.tile([128, CIN], F32, tag="cp")
                ps = pst[:, :NT392]
                for t, (dy, dx) in enumerate(taps):
                    rhs = a1pad[:, n, h0+dy:h0+dy+14, dx:dx+28]
                    nc.tensor.matmul(ps[:], w2hsb[:, t, :], rhs,
                                     start=(t == 0), stop=(t == 8))
                nc.scalar.activation(ygrp[:, si*NT392:(si+1)*NT392], ps[:], AFT.Relu,
                                     bias=bn2b[:], scale=bn2s[:])
            for s0, npx in chunks:
                nb = npx // 32
                rmax = dsm.tile([128, 49], F32, tag="rmax")
                nc.vector.tensor_reduce(
                    rmax[:, :nb],
                    ygrp[:, s0:s0+npx].rearrange("p (b j) -> p b j", b=nb, j=32),
                    axis=mybir.AxisListType.X, op=AL.max, apply_transpose=True)
                emit_expmask(rmax[:, :nb])
                dcm = dsm.tile([128, GRP], F32, tag="dcm")
                nc.vector.transpose(dcm[:, :npx],
                                    rmax[:, :nb].unsqueeze(2).broadcast_to([128, nb, 32]))
                nc.vector._custom_dve(
                    BFP_QUANT_ANT,
                    out=a2[:, 2*g*HW + s0:2*g*HW + s0 + npx],
                    in0=ygrp[:, s0:s0+npx],
                    in1=dcm[:, :npx],
                    s0=EXP_S0, s1=EXP_S1,
                )

        def emit_l3(t0, gn):
            nf = gn * CIN
            na = nf // 32          # 32-channel blocks in this emit
            xh3 = xt3.tile([128, 5 * CIN], BF16, tag="xh3")
            nc.sync.dma_start(xh3[:, :nf].rearrange("p (j c) -> p j c", j=gn, c=CIN),
                              xTh[128*t0:128*t0 + 128*gn, :].rearrange("(j p) c -> p j c", p=128))
            y3b = y3p.tile([128, 5 * CIN], BF16, tag="y3b")
            for j in range(gn):
                ps3 = p3p.tile([128, CIN], F32, tag="c3g")
                a2t = a2[:, 128*(t0+j):128*(t0+j+1)]
                nc.tensor.matmul(ps3[:], a2t, w3hsb[:], start=True, stop=False)
                nc.tensor.matmul(ps3[:], identsb[:], xh3[:, j*CIN:(j+1)*CIN],
                                 start=False, stop=True)
                nc.scalar.activation(y3b[:, j*CIN:(j+1)*CIN], ps3[:], AFT.Relu)
            # block max: two bf16 2x-mode tensor_max levels, then a short reduce
            yv = y3b[:, :nf].rearrange("p (a k) -> p a k", k=32)
            t1 = tp.tile([128, 1280], BF16, tag="t1")
            t1v = t1[:].rearrange("p (a k) -> p a k", k=16)[:, :na, :]
            nc.vector.tensor_max(t1v, yv[:, :, 0:16], yv[:, :, 16:32])
            t2 = tp.tile([128, 640], BF16, tag="t2")
            t2v = t2[:].rearrange("p (a k) -> p a k", k=8)[:, :na, :]
            nc.vector.tensor_max(t2v, t1v[:, :, 0:8], t1v[:, :, 8:16])
            rm3 = dsm.tile([128, 80], F32, tag="rm3")
            nc.vector.tensor_reduce(rm3[:, :na], t2v,
                                    axis=mybir.AxisListType.X, op=AL.max)
            emit_expmask(rm3[:, :na])
            o3 = stage.tile([128, 5 * CIN], BF16, tag="o3")
            nc.vector._custom_dve(
                BFP_QUANT_ANT,
                out=o3[:, :nf].rearrange("p (a k) -> p a k", k=32),
                in0=yv,
                in1=rm3[:, :na].unsqueeze(2).broadcast_to([128, na, 32]),
                s0=EXP_S0, s1=EXP_S1,
            )
            nc.sync.dma_start(outT[128*t0:128*t0 + 128*gn, :].rearrange("(j p) c -> p j c", p=128),
                              o3[:, :nf].rearrange("p (j c) -> p j c", j=gn, c=CIN))

        # ================= interleaved schedule =================
        emit_xload(0, split=True)
        for n in range(1, 4):
            emit_xload(n)
        # non-critical params go behind the first input slabs in the sync queue
        w2hsb = wp.tile([128, 9, WID], BF16)
        nc.sync.dma_start(w2hsb[:], w2h[:].rearrange("t c o -> c t o"))
        bn2s = wp.tile([128, 1], F32); nc.sync.dma_start(bn2s[:], inv2[:])
        bn2b = wp.tile([128, 1], F32); nc.sync.dma_start(bn2b[:], bet2[:])
        w3hsb = wp.tile([128, CIN], BF16)
        nc.sync.dma_start(w3hsb[:], w3h[:])
        identsb = wp.tile([128, 128], BF16)
        nc.sync.dma_start(identsb[:], ident[:])
        emit_l1(0, chunks=((0, 1120), (1120, 448)))
        emit_xload(4); emit_xload(5)
        emit_l1(1)
        emit_l2(0)
        emit_xload(6); emit_xload(7)
        emit_l1(2)
        for t0 in (0, 4, 8):        # tiles 0-11: needs quant2(0) only
            emit_l3(t0, 4)
        emit_l2(1)
        emit_l1(3)
        for t0 in (12, 16, 20):     # tiles 12-23: needs quant2(1)
            emit_l3(t0, 4)
        emit_l2(2)
        for t0 in (24, 28, 32):     # tiles 24-35: needs quant2(2)
            emit_l3(t0, 4)
        emit_l2(3, chunks=((0, 1024), (1024, 544)))
        for t0 in (36, 40):         # tiles 36-43: needs quant2(3) chunk A
            emit_l3(t0, 4)
        emit_l3(44, 4)              # tiles 44-48: needs quant2(3) chunk B
        emit_l3(48, 1)

    nc.finalize()
    return nc


# ---------------- host-side parameter prep ---------------------------------
def _w_quant_np(w, blk=32):
    O, I, kh, kw = w.shape
    wb = w.reshape(O, I // blk, blk, kh, kw)
    alpha = np.maximum(np.abs(wb).max(axis=2, keepdims=True) / np.float32(127.0),
                       np.float32(1e-24)).astype(np.float32)
    q = (np.round(wb / alpha) * alpha).astype(np.float32)
    return q.reshape(O, I, kh, kw)


def _bn_fold(g, b, m, v):
    inv = (g / np.sqrt(v + np.float32(1e-5))).astype(np.float32)
    beta = (b - m * inv).astype(np.float32)
    return inv, beta


def _split_bf16(a):
    hi = a.astype(ml_dtypes.bfloat16)
    lo = (a - hi.astype(np.float32)).astype(ml_dtypes.bfloat16)
    return hi, lo


_NC_CACHE = {}

def kernel(x, w1, w2, w3,
           bn1_g, bn1_b, bn1_m, bn1_v,
           bn2_g, bn2_b, bn2_m, bn2_v,
           bn3_g, bn3_b, bn3_m, bn3_v,
           _want_trace=False):
    x = np.asarray(x, np.float32)
    w1q = _w_quant_np(np.asarray(w1, np.float32))
    w2q = _w_quant_np(np.asarray(w2, np.float32))
    w3q = _w_quant_np(np.asarray(w3, np.float32))
    inv1, bet1 = _bn_fold(*[np.asarray(a, np.float32) for a in (bn1_g, bn1_b, bn1_m, bn1_v)])
    inv2, bet2 = _bn_fold(*[np.asarray(a, np.float32) for a in (bn2_g, bn2_b, bn2_m, bn2_v)])
    inv3, bet3 = _bn_fold(*[np.asarray(a, np.float32) for a in (bn3_g, bn3_b, bn3_m, bn3_v)])

    # bn3 beta folded into the residual input; conv1 bias corrected for it
    xb3 = (x + bet3[None, :, None, None]).astype(np.float32)
    K = (w1q[:, :, 0, 0].astype(np.float64) @ bet3.astype(np.float64))
    bet1c = (bet1.astype(np.float64) - inv1.astype(np.float64) * K).astype(np.float32)

    w1T = np.ascontiguousarray(w1q[:, :, 0, 0].T)                     # [512, 128]
    w2T = np.ascontiguousarray(w2q.transpose(2, 3, 1, 0).reshape(9, WID, WID))
    w3f = (w3q[:, :, 0, 0] * inv3[:, None]).astype(np.float32)
    w3T = np.ascontiguousarray(w3f.T)                                 # [128, 512]

    w1hT, w1lT = _split_bf16(w1T)
    w2hT = w2T.astype(ml_dtypes.bfloat16)
    w3hT = w3T.astype(ml_dtypes.bfloat16)

    xb3v = xb3.reshape(64, CIN, HW)
    xh_ = xb3v.astype(ml_dtypes.bfloat16)                             # [64, 512, 784]
    # residual uses the same hi rounding, transposed to pixel-major
    xTh_ = np.ascontiguousarray(xh_.transpose(0, 2, 1))               # [64, 784, 512]

    if "nc" not in _NC_CACHE:
        _NC_CACHE["nc"] = build_nc()
    nc = _NC_CACHE["nc"]

    shared = dict(
        w1h=w1hT, w1l=w1lT, w2h=w2hT, w3h=w3hT,
        ident=np.eye(128, dtype=ml_dtypes.bfloat16),
        inv1=inv1.reshape(WID, 1), bet1=bet1c.reshape(WID, 1),
        inv2=inv2.reshape(WID, 1), bet2=bet2.reshape(WID, 1),
    )
    in_maps = []
    for c in range(8):
        m = dict(shared)
        m["xh"] = np.ascontiguousarray(xh_[8*c:8*(c+1)])
        m["xTh"] = np.ascontiguousarray(xTh_[8*c:8*(c+1)].reshape(PIX, CIN))
        in_maps.append(m)

    res = run_bass_kernel_spmd(nc, in_maps, list(range(8)), trace=_want_trace)
    out = np.empty((64, CIN, H, W), np.float32)
    for c in range(8):
        oT = res.results[c]["outT"].astype(np.float32).reshape(N_IMG, HW, CIN)
        out[8*c:8*(c+1)] = oT.transpose(0, 2, 1).reshape(N_IMG, CIN, H, W)
    if _want_trace:
        return out, res
    return out
